# revision 23
# baseline (speedup 1.0000x reference)
"""Multi-head sigmoid self-attention on 8 Trainium2 NeuronCores.

Sharding: pure data parallel — batch (8) split one element per core.
Per core (feature-major "T" = transposed layouts, all matmuls bf16):

  q^T = Wq @ x^T, k^T = Wk @ x^T   (feature-major)
  v   = x @ Wv^T                   (token-major)
  S^T = sigmoid((k_h q_h^T)/sqrt(F) + bias)  per head; two heads of a
        pair computed concurrently on the PE (row-packed K=64)
  attn^T = v_h^T @ S_h^T           (col-packed pairs of heads)
  o   = attn @ Wo^T + bo           (token-major)

The Scalar engine (sigmoid, ~111us busy) is the bottleneck; the PE
(~104us) is second. Emission is a fine-grained software pipeline paced
around the Scalar engine: each score group (one key tile, two heads,
2 matmuls + 1 sigmoid) is followed by one attention step (lag-1
iteration) and ~550ns of projection filler, so sigmoids flow
back-to-back while projections absorb the PE slack. Input DMAs are
spread across three queues (sync/gpsimd/scalar HWDGE+SWDGE) so all
weights land by ~12us, and a few warmup matmuls lift the PE clock
gate before real work arrives.
"""

import os
import sys

import numpy as np

if "/opt/trn_rl_repo" not in sys.path:
    sys.path.insert(0, "/opt/trn_rl_repo")

P = 128
F = 768
N = 1024
H = 12
HD = 64
KO = F // P      # 6 feature stripes
NT = N // P      # 8 token tiles
CH = 2           # 512-token chunks
CW = N // CH     # 512
HP = H // 2      # 6 head pairs
FW = F // 2      # 384 (psum-sized feature chunk)
SCALE = 1.0 / float(np.sqrt(np.float32(F)))

_CACHE = {}

LAST_EXEC_NS = None


def _build():
    import concourse.mybir as mybir
    import concourse.tile as tile
    from concourse import bacc

    f32 = mybir.dt.float32
    bf16 = mybir.dt.bfloat16
    f8 = mybir.dt.float8e4
    ADD = mybir.AluOpType.add
    SIG = mybir.ActivationFunctionType.Sigmoid
    DR = mybir.MatmulPerfMode.DoubleRow

    nc = bacc.Bacc("TRN2", target_bir_lowering=False, debug=False)

    xT_d = nc.dram_tensor("xT", [P, KO, N], bf16, kind="ExternalInput").ap()
    wq_d = nc.dram_tensor("wqT", [P, KO // 2, 2, F], f8,
                          kind="ExternalInput").ap()
    wk_d = nc.dram_tensor("wkT", [P, KO // 2, 2, F], f8,
                          kind="ExternalInput").ap()
    x8_d = nc.dram_tensor("x8", [P, KO // 2, 2, N], f8,
                          kind="ExternalInput").ap()
    wv_d = nc.dram_tensor("wvT", [P, KO, F], bf16, kind="ExternalInput").ap()
    wo_d = nc.dram_tensor("woT", [P, KO, F], bf16, kind="ExternalInput").ap()
    bq_d = nc.dram_tensor("bqs", [P, KO], f32, kind="ExternalInput").ap()
    bk_d = nc.dram_tensor("bks", [P, KO], f32, kind="ExternalInput").ap()
    bv_d = nc.dram_tensor("bv1", [1, F], f32, kind="ExternalInput").ap()
    bo_d = nc.dram_tensor("bo1", [1, F], f32, kind="ExternalInput").ap()
    bi_d = nc.dram_tensor("bir", [P, 1], f32, kind="ExternalInput").ap()
    o_d = nc.dram_tensor("o", [N, F], f32, kind="ExternalOutput").ap()

    with tile.TileContext(nc) as tc:
        with (
            tc.tile_pool(name="sb", bufs=1) as sb,
            tc.tile_pool(name="ps", bufs=1, space="PSUM") as psp,
        ):
            # ---- persistent SBUF tensors -------------------------------
            xT = sb.tile([P, KO, N], bf16, tag="xT")
            wq = sb.tile([P, KO // 2, 2, F], f8, tag="wq")
            wk = sb.tile([P, KO // 2, 2, F], f8, tag="wk")
            x8 = sb.tile([P, KO // 2, 2, N], f8, tag="x8")
            wv = sb.tile([P, KO, F], bf16, tag="wv")
            wo = sb.tile([P, KO, F], bf16, tag="wo")
            qT = sb.tile([P, KO, N], bf16, tag="qT")
            kT = sb.tile([P, KO, N], bf16, tag="kT")
            v_sb = sb.tile([P, NT, F], bf16, tag="v")
            attnT = sb.tile([P, KO, N], bf16, tag="attnT")
            bqs = sb.tile([P, KO], f32, tag="bqs")
            bks = sb.tile([P, KO], f32, tag="bks")
            bvr = sb.tile([P, F], f32, tag="bvr")
            bor = sb.tile([P, F], f32, tag="bor")
            bir = sb.tile([P, 1], f32, tag="bir")
            bv1 = sb.tile([1, F], f32, tag="bv1")
            bo1 = sb.tile([1, F], f32, tag="bo1")
            ones = sb.tile([1, P], f32, tag="ones")

            # ---- input DMAs on three queues ----------------------------
            # One big transfer per tensor: consumers wait for the whole
            # batch anyway (coarse DMA-lane semaphores), and keeping the
            # HWDGE DMA count <= 8 avoids cross-queue semaphore-lane
            # collisions between the sync and scalar rings.
            # sync (fastest ring): the first-sigmoid critical path —
            # fp8 k/q weights and the fp8 activations, 1.9MB total.
            # All large inputs ride the sync ring in need-order; no
            # SWDGE input DMAs (the gpsimd queue's trailing DRAIN was
            # observed to gate the whole machine for ~17us).
            nc.sync.dma_start(x8[:], x8_d[:])
            nc.sync.dma_start(wk[:], wk_d[:])
            nc.sync.dma_start(wq[:], wq_d[:])
            nc.sync.dma_start(xT[:], xT_d[:])
            nc.sync.dma_start(wv[:], wv_d[:])
            nc.sync.dma_start(wo[:], wo_d[:])
            # scalar (HWDGE on ACT): tiny bias tensors.
            nc.scalar.dma_start(bks[:], bk_d)
            nc.scalar.dma_start(bqs[:], bq_d)
            nc.scalar.dma_start(bir[:], bi_d)
            nc.scalar.dma_start(bv1[:], bv_d)
            nc.scalar.dma_start(bo1[:], bo_d)

            # ---- PE warmup ---------------------------------------------
            # A few matmuls on a zeroed scratch tile keep the PE busy
            # while input DMAs land, so the HAM clock gate opens before
            # real work starts. Result is never read.
            warm = sb.tile([P, CW], bf16, tag="warm")
            nc.vector.memset(warm[:], 0.0)
            wps = psp.tile([P, CW], f32, tag="pp", bufs=3, name="warm")
            NWARM = 10
            for i in range(NWARM):
                nc.tensor.matmul(wps[:], warm[:, 0:P], warm[:],
                                 start=(i == 0), stop=(i == NWARM - 1))

            # ---- emission helpers --------------------------------------
            def gen_qk_stripe(mo):
                """Generator: q^T/k^T projections for feature stripe mo.
                Yields estimated PE ns after each unit so the feeder can
                pace emission. Both token chunks accumulate side by side
                (one weight load per two matmuls). k before q: scores
                need the full kT stripe but only one qT chunk."""
                for w_sb, bst, dst in ((wk, bks, kT), (wq, bqs, qT)):
                    ps = [psp.tile([P, CW], f32, tag="pp", bufs=3,
                                   name="ps_qk") for _ in range(CH)]
                    for ko2 in range(KO // 2):
                        for ch in range(CH):
                            nc.tensor.matmul(
                                ps[ch][:],
                                w_sb[:, ko2, :, mo * P:(mo + 1) * P],
                                x8[:, ko2, :, ch * CW:(ch + 1) * CW],
                                start=(ko2 == 0), stop=(ko2 == KO // 2 - 1),
                                perf_mode=DR,
                            )
                            yield 260
                    for ch in range(CH):
                        nc.vector.tensor_tensor(
                            dst[:, mo, ch * CW:(ch + 1) * CW], ps[ch][:],
                            bst[:, mo:mo + 1].to_broadcast([P, CW]), ADD,
                        )
                    yield 80

            def gen_v(trange, j):
                """Generator: v projection (token-major) for token tiles
                in trange, feature chunk j."""
                for t in trange:
                    ps = psp.tile([P, CW], f32, tag="pp", bufs=3, name="ps_v")
                    psv = ps[:, 0:FW]
                    for ko in range(KO):
                        nc.tensor.matmul(
                            psv,
                            xT[:, ko, t * P:(t + 1) * P],
                            wv[:, ko, j * FW:(j + 1) * FW],
                            start=(ko == 0), stop=(ko == KO - 1),
                        )
                        yield 165
                    nc.vector.tensor_tensor(
                        v_sb[:, t, j * FW:(j + 1) * FW], psv,
                        bvr[:, j * FW:(j + 1) * FW], ADD,
                    )
                    yield 80

            def gen_oproj(ch):
                """Generator: output projection for the 4 token tiles of
                chunk ch. Both feature chunks accumulate side by side so
                each attnT tile (the stationary operand) is loaded once
                for two matmuls."""
                for tt in range(4):
                    tg = ch * 4 + tt
                    op = sb.tile([P, F], f32, tag="osb", bufs=2, name="osb")
                    ps = [psp.tile([P, CW], f32, tag="pp", bufs=3,
                                   name="ps_o") for _ in range(2)]
                    for ko in range(KO):
                        for j in range(2):
                            nc.tensor.matmul(
                                ps[j][:, 0:FW],
                                attnT[:, ko, tg * P:(tg + 1) * P],
                                wo[:, ko, j * FW:(j + 1) * FW],
                                start=(ko == 0), stop=(ko == KO - 1),
                            )
                            yield 180
                    for j in range(2):
                        nc.vector.tensor_tensor(
                            op[:, j * FW:(j + 1) * FW], ps[j][:, 0:FW],
                            bor[:, j * FW:(j + 1) * FW], ADD,
                        )
                        yield 80
                    nc.sync.dma_start(o_d[tg * P:(tg + 1) * P, :], op[:])

            def emit_score_group(ch, hp, kt):
                """Scores for both heads of pair hp, query chunk ch, key
                tile kt: two row-packed matmuls + one sigmoid."""
                qsl = slice(ch * CW, (ch + 1) * CW)
                ksl = slice(kt * P, (kt + 1) * P)
                sc = psp.tile([P, 2, CW], f32, tag="sc", bufs=2, name="sc")
                nc.tensor.matmul(sc[:, 0, :], kT[0:64, hp, ksl],
                                 qT[0:64, hp, qsl], start=True, stop=True)
                nc.tensor.matmul(sc[:, 1, :], kT[64:128, hp, ksl],
                                 qT[64:128, hp, qsl], start=True, stop=True)
                st = sb.tile([P, 2, CW], bf16, tag="st", bufs=26, name="st")
                nc.scalar.activation(st[:], sc[:], SIG,
                                     bias=bir[:, 0:1], scale=SCALE / 256.0)
                return st

            def gen_attn(ch, hp, units):
                """Generator: attn^T accumulation for head pair hp over
                the 8 key tiles (one yield per key tile), then copy out
                to attnT. units = 16 (st_tile, slot) refs."""
                qsl = slice(ch * CW, (ch + 1) * CW)
                at = psp.tile([P, CW], f32, tag="at", bufs=1, name="at")
                for kt in range(NT):
                    st = units[kt]
                    nc.tensor.matmul(at[0:64, :],
                                     v_sb[:, kt, hp * P:hp * P + HD],
                                     st[:, 0, :],
                                     start=(kt == 0), stop=(kt == NT - 1))
                    nc.tensor.matmul(at[64:128, :],
                                     v_sb[:, kt, hp * P + HD:(hp + 1) * P],
                                     st[:, 1, :],
                                     start=(kt == 0), stop=(kt == NT - 1))
                    yield 230
                nc.vector.tensor_copy(attnT[:, hp, qsl], at[:])

            def run_gen(g):
                for _ in g:
                    pass

            # ---- feeder ------------------------------------------------
            # Ordered queue of (name, generator). Emission order IS the
            # dependency order under Tile, so consumers must require()
            # their producer's generator to be fully emitted first.
            queue = []

            def feed(budget):
                # Stop before overshooting: a typical step is ~200ns of
                # PE time, and every ns past the sigmoid cadence starves
                # the Scalar engine.
                spent = 0
                while queue and spent + 200 <= budget:
                    try:
                        spent += next(queue[0][1])
                    except StopIteration:
                        queue.pop(0)
                return spent

            def require(name):
                while any(n == name for n, _ in queue):
                    try:
                        next(queue[0][1])
                    except StopIteration:
                        queue.pop(0)

            # ---- software-pipelined emission ---------------------------
            # Prologue: stripe 0 of q/k (scores' first input) and half of
            # v's first feature chunk run before the main loop.
            # Broadcast bv/bo rows across all 128 partitions with a
            # K=1 matmul against a column of ones (saves two 384KB
            # DMAs); paced through the filler queue so the fp32 matmuls
            # stay off the first-sigmoid critical path.
            nc.vector.memset(ones[:], 1.0)

            def gen_bbc():
                for b1, brow in ((bv1, bvr), (bo1, bor)):
                    bps = psp.tile([P, CW], f32, tag="pp", bufs=3,
                                   name="bbc")
                    for j in range(2):
                        nc.tensor.matmul(bps[:, 0:FW], ones[:],
                                         b1[:, j * FW:(j + 1) * FW],
                                         start=True, stop=True)
                        yield 550
                        nc.vector.tensor_copy(
                            brow[:, j * FW:(j + 1) * FW], bps[:, 0:FW])
                        yield 80

            run_gen(gen_qk_stripe(0))
            queue.extend([
                ("bbc", gen_bbc()),
                ("qk1", gen_qk_stripe(1)),
                ("v0a", gen_v(range(0, 4), 0)),
                ("v0", gen_v(range(4, 8), 0)),
                ("qk2", gen_qk_stripe(2)),
                ("v1a", gen_v(range(0, 4), 1)),
                ("qk3", gen_qk_stripe(3)),
                ("qk4", gen_qk_stripe(4)),
                ("qk5", gen_qk_stripe(5)),
                ("v1", gen_v(range(4, 8), 1)),
            ])

            # (0,5) sits at index 8 so chunk-0 attention finishes early
            # enough for its output projection to run inside the loop.
            seq = [(0, 0), (1, 0), (0, 1), (1, 1), (0, 2), (1, 2),
                   (0, 3), (0, 4), (0, 5), (1, 3), (1, 4), (1, 5)]

            pending = []
            for it, (ch, hp) in enumerate(seq):
                require(f"qk{hp}")
                ag = aprev = None
                if len(pending) == 2:
                    aprev = pending.pop(0)
                    require("v0" if aprev[1] < 3 else "v1")
                    ag = gen_attn(*aprev)
                # Late iterations push harder so the chunk-0 output
                # projection finishes inside the sigmoid window instead
                # of trailing it.
                fill = 760 if ag is None else (900 if it >= 9 else 560)
                units = []
                for kt in range(NT):
                    units.append(emit_score_group(ch, hp, kt))
                    if ag is not None:
                        next(ag, None)
                    feed(fill)
                if ag is not None:
                    run_gen(ag)
                    if (aprev[0], aprev[1]) == (0, HP - 1):
                        queue.append(("op0", gen_oproj(0)))
                pending.append((ch, hp, units))

            # Epilogue: last two attentions, chunk-0 oproj leftovers,
            # then chunk-1 oproj.
            for pch, php, punits in pending:
                require("v0" if php < 3 else "v1")
                ag = gen_attn(pch, php, punits)
                for kt in range(NT):
                    next(ag, None)
                    feed(900)
                run_gen(ag)
                if (pch, php) == (0, HP - 1):
                    queue.append(("op0", gen_oproj(0)))
            queue.append(("op1", gen_oproj(1)))
            while queue:
                feed(1 << 30)

    nc.compile()
    return nc


def _bf16(a):
    import ml_dtypes
    return np.ascontiguousarray(a).astype(ml_dtypes.bfloat16)


def _prep_w(W):
    W = np.asarray(W, dtype=np.float32)
    return _bf16(W.T.reshape(KO, P, F).transpose(1, 0, 2))


def _prep_w8(W):
    """fp8(e4m3) DoubleRow weight prep for the q/k projections,
    pre-scaled by 16 so typical transformer-init weights sit in e4m3's
    normal range; the 16*16=256 factor on the scores is folded into
    the sigmoid scale. Layout [P, KO/2, 2, F]: stripe pairs packed for
    the PE's DoubleRow contraction."""
    import ml_dtypes
    W = np.asarray(W, dtype=np.float32) * 16.0
    W = W.T.reshape(KO // 2, 2, P, F).transpose(2, 0, 1, 3)
    return np.ascontiguousarray(W).astype(ml_dtypes.float8_e4m3)


def _prep_x8(xb):
    """fp8 activations for the q/k projections, DoubleRow layout
    [P, KO/2, 2, N] matching _prep_w8's stripe pairing."""
    import ml_dtypes
    xt = xb.T.reshape(KO // 2, 2, P, N).transpose(2, 0, 1, 3)
    return np.ascontiguousarray(xt).astype(ml_dtypes.float8_e4m3)


def kernel(x, bias, Wq, bq, Wk, bk, Wv, bv, Wo, bo):
    global LAST_EXEC_NS
    from concourse import bass_utils

    if "nc" not in _CACHE:
        _CACHE["nc"] = _build()
    nc = _CACHE["nc"]

    x = np.asarray(x, dtype=np.float32)
    shared = {
        "wqT": _prep_w8(Wq),
        "wkT": _prep_w8(Wk),
        "wvT": _prep_w(Wv),
        "woT": _prep_w(Wo),
        "bqs": np.ascontiguousarray(
            np.asarray(bq, np.float32).reshape(KO, P).T) * 16.0,
        "bks": np.ascontiguousarray(
            np.asarray(bk, np.float32).reshape(KO, P).T) * 16.0,
        "bv1": np.asarray(bv, np.float32).reshape(1, F).copy(),
        "bo1": np.asarray(bo, np.float32).reshape(1, F).copy(),
        "bir": np.full((P, 1), np.float32(np.asarray(bias)),
                       dtype=np.float32),
    }
    in_maps = []
    for b in range(x.shape[0]):
        m = dict(shared)
        m["xT"] = _bf16(x[b].T.reshape(KO, P, N).transpose(1, 0, 2))
        m["x8"] = _prep_x8(x[b])
        in_maps.append(m)

    trace = bool(os.environ.get("KERNEL_TRACE"))
    if trace:
        try:
            import ntff_hook
            ntff_hook.install()
        except Exception:
            trace = False

    res = bass_utils.run_bass_kernel_spmd(
        nc, in_maps, core_ids=list(range(len(in_maps))), trace=trace)
    LAST_EXEC_NS = res.exec_time_ns
    return np.stack([r["o"] for r in res.results]).astype(np.float32)
